# revision 2
# baseline (speedup 1.0000x reference)
"""Trainium2 Bass kernel for BERT word-pooling (segment mean + CLS).

Computation (matches the jax reference):
  hidden = mean over 4 layers of hidden_layers[4, B, T, D]
  per example b: word_emb[j] = mean of hidden[b, t] over tokens with
  word_ids[b, t] == j (j < 100; 100 is the pad sentinel), empty words -> 0
  output rows per example: [cls = hidden[b, 0], word_emb[0..99]]
  -> [B*101, D]

Strategy: pure data parallel, 4 examples per core across 8 cores.
Per example the segment-sum is a one-hot matmul on the tensor engine:
  psum[j, d] = sum_{l,t} S[t, j] * h[l, t, d]      (layer sum folded in)
  counts[j]  = sum_t S[t, j] * 4.0
  out[j, d]  = psum[j, d] / max(counts[j], 4)      (= segment mean / 4 layers)
The one-hot columns are shifted by +1 (word j -> column j+1) and column 0
marks token 0, so the CLS row falls out of the same matmul + scale
pipeline (its count is 1 -> scale 1/4) and rows 0..100 of the result tile
are exactly one example's output block.

Memory plan (the kernel is HBM-bound: 32 MB of reads per core):
  * Token -> (partition, column) map is p = t // 4, c = t % 4, so every
    h load is 16 KB fully contiguous per partition (one big descriptor)
    and the word-id load is a tiny contiguous [128, 4] strided read.
  * All 16 layer-example h tiles live in SBUF at once (128 KB/partition
    in f16), so all 16 load DMAs are issued up front with no buffer-reuse
    dependencies: the SWDGE queue streams 32 MB at line rate start to
    finish while the PE/DVE chase layer arrivals.
  * f32 -> f16 happens inline in the SWDGE DMAs (only path that casts);
    all matmuls run f16 (4x fp32 PE rate), PSUM accumulates in f32.
  * Stores go on the scalar HWDGE queue (own queue -> a store waiting on
    compute can't head-of-line-block any load), 101 rows per example.
"""

import sys

for _p in ("/opt/trn_rl_repo", "/opt/trn_rl_repo/concourse"):
    if _p not in sys.path:
        sys.path.append(_p)

from contextlib import ExitStack

import numpy as np

import concourse.bacc as bacc
import concourse.bass as bass
import concourse.tile as tile
from concourse import mybir
from concourse.bass_utils import run_bass_kernel_spmd

B, T, D, W = 32, 512, 1024, 100
N_CORES = 8
BL = B // N_CORES          # examples per core
NT = T // 128              # token columns per partition (p = t // 4)
ND = D // 512              # 512-wide d chunks (one PSUM bank each)
OUT_ROWS = BL * (W + 1)    # output rows per core

_f32 = mybir.dt.float32
_f16 = mybir.dt.float16
_i32 = mybir.dt.int32


def _build_program() -> bass.Bass:
    # Bacc (not raw Bass): its compile() runs generate_event_semaphores,
    # which splits multi-wait DMAs (DMA instrs have a single HW wait slot).
    nc = bacc.Bacc(
        "TRN2", target_bir_lowering=False, debug=False, num_devices=N_CORES
    )
    hid = nc.declare_dram_parameter("hidden", [4, BL, T, D], _f32, isOutput=False)
    wid = nc.declare_dram_parameter("wid", [BL, T], _i32, isOutput=False)
    out = nc.declare_dram_parameter("out", [OUT_ROWS, D], _f32, isOutput=True)

    with tile.TileContext(nc) as tc, ExitStack() as ctx:
        const = ctx.enter_context(tc.tile_pool(name="const", bufs=1))
        hres = ctx.enter_context(tc.tile_pool(name="hres", bufs=1))
        spool = ctx.enter_context(tc.tile_pool(name="spool", bufs=1))
        vpool = ctx.enter_context(tc.tile_pool(name="vpool", bufs=1))
        opool = ctx.enter_context(tc.tile_pool(name="opool", bufs=2))
        psum = ctx.enter_context(tc.tile_pool(name="psum", bufs=2, space="PSUM"))

        # column j holds value j-1 in every partition (f32: is_equal wants f32
        # operands). Word j then lands in one-hot column j+1, and column 0
        # (value -1, never a word id) is reserved for the CLS marker, so the
        # out_sb rows 0..100 are exactly one example's output block.
        iota_i = const.tile([128, 128], _i32)
        nc.gpsimd.iota(iota_i[:], [[1, 128]], base=-1, channel_multiplier=0)
        iota_t = const.tile([128, 128], _f32)
        nc.vector.tensor_copy(iota_t[:], iota_i[:])
        # counts rhs: 4.0 so counts come out as 4*count (the layer factor)
        ones4 = const.tile([128, 1], _f16)
        nc.vector.memset(ones4[:], 4.0)

        # word ids for all examples up front, one tiny contiguous read per
        # example: widt[p, 4b+c] = wid[b, 4p+c]
        widt = vpool.tile([128, BL * NT], _i32, tag="widt")
        for b in range(BL):
            nc.sync.dma_start(
                widt[:, b * NT : (b + 1) * NT],
                wid[b].rearrange("(p c) -> p c", p=128),
            )
        widt_f = vpool.tile([128, BL * NT], _f32, tag="widt_f")
        nc.vector.tensor_copy(widt_f[:], widt[:])

        # ALL h loads issued before any compute: 16 x 2MB SWDGE DMAs with
        # inline f32->f16 cast into fully-resident tiles. Example-major so
        # example b's four layers land consecutively.
        h_tiles = [[None] * 4 for _ in range(BL)]
        for b in range(BL):
            for l in range(4):
                h_bl = hres.tile(
                    [128, NT, D], _f16, tag=f"h_{b}_{l}", name=f"h_{b}_{l}"
                )
                nc.gpsimd.dma_start(
                    h_bl[:], hid[l, b].rearrange("(p c) m -> p c m", p=128)
                )
                h_tiles[b][l] = h_bl

        # one-hot S tiles for all examples: S[p, j] = (wid[4p+c] == j-1)
        s_tiles = [[None] * NT for _ in range(BL)]
        for b in range(BL):
            for c in range(NT):
                s_c = spool.tile([128, 128], _f16, tag=f"s_{b}_{c}", name=f"s_{b}_{c}")
                nc.vector.tensor_scalar(
                    s_c[:], iota_t[:], widt_f[:, b * NT + c : b * NT + c + 1], None,
                    mybir.AluOpType.is_equal,
                )
                if c == 0:
                    # CLS marker: token 0 (p=0, c=0) also feeds output row 0
                    nc.vector.memset(s_c[0:1, 0:1], 1.0)
                s_tiles[b][c] = s_c

        # counts + reciprocal scales for all examples early (PE and DVE are
        # otherwise idle while the first h tiles stream in)
        recips = []
        for b in range(BL):
            counts_ps = psum.tile([128, 1], _f32, tag="counts", bufs=2)
            for c in range(NT):
                nc.tensor.matmul(
                    counts_ps[:], s_tiles[b][c][:], ones4[:],
                    start=(c == 0), stop=(c == NT - 1),
                )
            scale_t = vpool.tile([128, 1], _f32, tag=f"scale_{b}")
            recip_t = vpool.tile([128, 1], _f32, tag=f"recip_{b}")
            nc.vector.tensor_scalar_max(scale_t[:], counts_ps[:], 4.0)
            nc.vector.reciprocal(recip_t[:], scale_t[:])
            recips.append(recip_t)

        # per example: 32 accumulating f16 matmuls (4 layers x 4 columns x
        # 2 d-chunks), layer-major so compute chases the DMA arrival order
        for b in range(BL):
            ps = [
                psum.tile([128, 512], _f32, tag=f"ps{d}", name=f"ps{d}", bufs=2)
                for d in range(ND)
            ]
            for l in range(4):
                for c in range(NT):
                    for d in range(ND):
                        nc.tensor.matmul(
                            ps[d][:],
                            s_tiles[b][c][:],
                            h_tiles[b][l][:, c, d * 512 : (d + 1) * 512],
                            start=(l == 0 and c == 0),
                            stop=(l == 3 and c == NT - 1),
                        )
            out_sb = opool.tile([128, D], _f32, tag="out_sb", name="out_sb")
            for d in range(ND):
                dsl = slice(d * 512, (d + 1) * 512)
                nc.vector.tensor_scalar(
                    out_sb[:, dsl], ps[d][:], recips[b][:, 0:1], None,
                    mybir.AluOpType.mult,
                )
            # 101-row store on the scalar HWDGE ring: stores never share a
            # queue with loads, so a store waiting on compute blocks nothing
            nc.scalar.dma_start(
                out[b * (W + 1) : (b + 1) * (W + 1), :], out_sb[0 : W + 1, :]
            )

    nc.compile()
    return nc


_PROGRAM = None
LAST_RESULTS = None   # BassKernelResults of the most recent run (for test.py)
TRACE = False         # set True from test.py to capture an NTFF profile


def _get_program() -> bass.Bass:
    global _PROGRAM
    if _PROGRAM is None:
        _PROGRAM = _build_program()
    return _PROGRAM


def kernel(hidden_layers, word_ids, num_words=W, **_ignored) -> np.ndarray:
    global LAST_RESULTS
    hidden_layers = np.asarray(hidden_layers, dtype=np.float32)
    word_ids = np.asarray(word_ids, dtype=np.int32)
    assert hidden_layers.shape == (4, B, T, D), hidden_layers.shape
    assert word_ids.shape == (B, T), word_ids.shape
    assert int(num_words) == W, num_words

    in_maps = []
    for i in range(N_CORES):
        sl = slice(i * BL, (i + 1) * BL)
        in_maps.append(
            {
                "hidden": np.ascontiguousarray(hidden_layers[:, sl]),
                "wid": np.ascontiguousarray(word_ids[sl]),
            }
        )

    res = run_bass_kernel_spmd(
        _get_program(), in_maps, core_ids=list(range(N_CORES)), trace=TRACE
    )
    LAST_RESULTS = res
    outs = [res.results[i]["out"] for i in range(N_CORES)]
    return np.concatenate(outs, axis=0)


# revision 6
# speedup vs baseline: 1.2559x; 1.2559x over previous
"""Trainium2 Bass kernel for BERT word-pooling (segment mean + CLS).

Computation (matches the jax reference):
  hidden = mean over 4 layers of hidden_layers[4, B, T, D]
  per example b: word_emb[j] = mean of hidden[b, t] over tokens with
  word_ids[b, t] == j (j < 100; 100 is the pad sentinel), empty words -> 0
  output rows per example: [cls = hidden[b, 0], word_emb[0..99]]
  -> [B*101, D]

Strategy: pure data parallel, 4 examples per core across 8 cores.
Per example the segment-sum is a one-hot matmul on the tensor engine:
  psum[j, d] = sum_{l,t} S[t, j] * h[l, t, d]      (layer sum folded in)
  counts[j]  = sum_t S[t, j] * 4.0
  out[j, d]  = psum[j, d] / max(counts[j], 4)      (= segment mean / 4 layers)
The one-hot columns are shifted by +1 (word j -> column j+1) and column 0
marks token 0, so the CLS row falls out of the same matmul + scale
pipeline (its count is 1 -> scale 1/4) and rows 0..100 of the result tile
are exactly one example's output block.

Memory plan (the kernel is HBM-bound: 32 MB of reads per core):
  * Token -> (partition, column) map is p = t // 4, c = t % 4, so every
    h load is 16 KB fully contiguous per partition (one big descriptor)
    and the word-id load is a tiny contiguous [128, 4] strided read.
  * All 16 layer-example h tiles live in SBUF at once (128 KB/partition
    in f16), so all 16 load DMAs are issued up front with no buffer-reuse
    dependencies: the SWDGE queue streams 32 MB at line rate start to
    finish while the PE/DVE chase layer arrivals.
  * f32 -> f16 happens inline in the SWDGE DMAs (only path that casts);
    all matmuls run f16 (4x fp32 PE rate), PSUM accumulates in f32.
  * Stores go on the scalar HWDGE queue (own queue -> a store waiting on
    compute can't head-of-line-block any load), 101 rows per example.
"""

import sys

for _p in ("/opt/trn_rl_repo", "/opt/trn_rl_repo/concourse"):
    if _p not in sys.path:
        sys.path.append(_p)

from contextlib import ExitStack

import numpy as np

import concourse.bacc as bacc
import concourse.bass as bass
import concourse.tile as tile
from concourse import mybir
from concourse.bass_utils import run_bass_kernel_spmd

B, T, D, W = 32, 512, 1024, 100
N_CORES = 8
BL = B // N_CORES          # examples per core
NT = T // 128              # token columns per partition (p = t // 4)
ND = D // 512              # 512-wide d chunks (one PSUM bank each)
OUT_PAD = 128              # padded per-example output rows: a 128-partition
                           # store spreads across all 16 SDMA engines, a
                           # 101-partition store lands on ONE engine (5 GB/s)
OUT_ROWS = BL * OUT_PAD    # output rows per core (kernel-side, padded)

_f32 = mybir.dt.float32
_f16 = mybir.dt.float16
_i32 = mybir.dt.int32


def _build_program() -> bass.Bass:
    # Bacc (not raw Bass): its compile() runs generate_event_semaphores,
    # which splits multi-wait DMAs (DMA instrs have a single HW wait slot).
    nc = bacc.Bacc(
        "TRN2", target_bir_lowering=False, debug=False, num_devices=N_CORES
    )
    hid = nc.declare_dram_parameter("hidden", [4, BL, T, D], _f32, isOutput=False)
    wid = nc.declare_dram_parameter("wid", [BL, T], _i32, isOutput=False)
    out = nc.declare_dram_parameter("out", [OUT_ROWS, D], _f32, isOutput=True)

    with tile.TileContext(nc) as tc, ExitStack() as ctx:
        const = ctx.enter_context(tc.tile_pool(name="const", bufs=1))
        hres = ctx.enter_context(tc.tile_pool(name="hres", bufs=1))
        spool = ctx.enter_context(tc.tile_pool(name="spool", bufs=1))
        vpool = ctx.enter_context(tc.tile_pool(name="vpool", bufs=1))
        opool = ctx.enter_context(tc.tile_pool(name="opool", bufs=2))
        psum = ctx.enter_context(tc.tile_pool(name="psum", bufs=2, space="PSUM"))

        # column j holds value j-1 in every partition (f32: is_equal wants f32
        # operands). Word j then lands in one-hot column j+1, and column 0
        # (value -1, never a word id) is reserved for the CLS marker, so the
        # out_sb rows 0..100 are exactly one example's output block.
        iota_i = const.tile([128, 128], _i32)
        nc.gpsimd.iota(iota_i[:], [[1, 128]], base=-1, channel_multiplier=0)
        iota_t = const.tile([128, 128], _f32)
        nc.vector.tensor_copy(iota_t[:], iota_i[:])
        # counts rhs: 4.0 so counts come out as 4*count (the layer factor)
        ones4 = const.tile([128, 1], _f16)
        nc.vector.memset(ones4[:], 4.0)

        # word ids for all examples up front, one tiny contiguous read per
        # example: widt[p, 4b+c] = wid[b, 4p+c]. These ride the scalar
        # HWDGE queue, which is otherwise idle until the first store: on a
        # queue shared with the h stream their 16B packets would only get
        # one round-robin turn each and complete ~15 us late.
        widt = vpool.tile([128, BL * NT], _i32, tag="widt")
        for b in range(BL):
            nc.scalar.dma_start(
                widt[:, b * NT : (b + 1) * NT],
                wid[b].rearrange("(p c) -> p c", p=128),
            )
        widt_f = vpool.tile([128, BL * NT], _f32, tag="widt_f")
        nc.vector.tensor_copy(widt_f[:], widt[:])

        # ALL h loads issued before any compute: 16 x 2MB SWDGE DMAs with
        # inline f32->f16 cast into fully-resident tiles. Example-major so
        # example b's four layers land consecutively.
        h_tiles = [[None] * 4 for _ in range(BL)]
        for b in range(BL):
            for l in range(4):
                h_bl = hres.tile(
                    [128, NT, D], _f16, tag=f"h_{b}_{l}", name=f"h_{b}_{l}"
                )
                nc.gpsimd.dma_start(
                    h_bl[:], hid[l, b].rearrange("(p c) m -> p c m", p=128)
                )
                h_tiles[b][l] = h_bl

        # one-hot S tiles for all examples: S[p, j] = (wid[4p+c] == j-1)
        s_tiles = [[None] * NT for _ in range(BL)]
        for b in range(BL):
            for c in range(NT):
                s_c = spool.tile([128, 128], _f16, tag=f"s_{b}_{c}", name=f"s_{b}_{c}")
                nc.vector.tensor_scalar(
                    s_c[:], iota_t[:], widt_f[:, b * NT + c : b * NT + c + 1], None,
                    mybir.AluOpType.is_equal,
                )
                if c == 0:
                    # CLS marker: token 0 (p=0, c=0) also feeds output row 0
                    nc.vector.memset(s_c[0:1, 0:1], 1.0)
                s_tiles[b][c] = s_c

        # counts + reciprocal scales for all examples early (PE and DVE are
        # otherwise idle while the first h tiles stream in)
        recips = []
        for b in range(BL):
            counts_ps = psum.tile([128, 1], _f32, tag="counts", bufs=2)
            for c in range(NT):
                nc.tensor.matmul(
                    counts_ps[:], s_tiles[b][c][:], ones4[:],
                    start=(c == 0), stop=(c == NT - 1),
                )
            scale_t = vpool.tile([128, 1], _f32, tag=f"scale_{b}")
            recip_t = vpool.tile([128, 1], _f32, tag=f"recip_{b}")
            nc.vector.tensor_scalar_max(scale_t[:], counts_ps[:], 4.0)
            nc.vector.reciprocal(recip_t[:], scale_t[:])
            recips.append(recip_t)

        # per example: 32 accumulating f16 matmuls (4 layers x 4 columns x
        # 2 d-chunks), layer-major so compute chases the DMA arrival order
        for b in range(BL):
            ps = [
                psum.tile([128, 512], _f32, tag=f"ps{d}", name=f"ps{d}", bufs=2)
                for d in range(ND)
            ]
            for l in range(4):
                for c in range(NT):
                    for d in range(ND):
                        nc.tensor.matmul(
                            ps[d][:],
                            s_tiles[b][c][:],
                            h_tiles[b][l][:, c, d * 512 : (d + 1) * 512],
                            start=(l == 0 and c == 0),
                            stop=(l == 3 and c == NT - 1),
                        )
            out_sb = opool.tile([128, D], _f32, tag="out_sb", name="out_sb")
            for d in range(ND):
                dsl = slice(d * 512, (d + 1) * 512)
                nc.vector.tensor_scalar(
                    out_sb[:, dsl], ps[d][:], recips[b][:, 0:1], None,
                    mybir.AluOpType.mult,
                )
            # full-128-partition store on the scalar HWDGE ring (no loads
            # behind it in that queue, so waiting on compute blocks nothing);
            # the host slices rows 0..100 of each example block
            nc.scalar.dma_start(out[b * OUT_PAD : (b + 1) * OUT_PAD, :], out_sb[:])

    nc.compile()
    return nc


_PROGRAM = None
LAST_RESULTS = None   # BassKernelResults of the most recent run (for test.py)
TRACE = False         # set True from test.py to capture an NTFF profile


def _get_program() -> bass.Bass:
    global _PROGRAM
    if _PROGRAM is None:
        _PROGRAM = _build_program()
    return _PROGRAM


def kernel(hidden_layers, word_ids, num_words=W, **_ignored) -> np.ndarray:
    global LAST_RESULTS
    hidden_layers = np.asarray(hidden_layers, dtype=np.float32)
    word_ids = np.asarray(word_ids, dtype=np.int32)
    assert hidden_layers.shape == (4, B, T, D), hidden_layers.shape
    assert word_ids.shape == (B, T), word_ids.shape
    assert int(num_words) == W, num_words

    in_maps = []
    for i in range(N_CORES):
        sl = slice(i * BL, (i + 1) * BL)
        in_maps.append(
            {
                "hidden": np.ascontiguousarray(hidden_layers[:, sl]),
                "wid": np.ascontiguousarray(word_ids[sl]),
            }
        )

    res = run_bass_kernel_spmd(
        _get_program(), in_maps, core_ids=list(range(N_CORES)), trace=TRACE
    )
    LAST_RESULTS = res
    # kernel output is padded to 128 rows per example; keep rows 0..100
    outs = [
        res.results[i]["out"].reshape(BL, OUT_PAD, D)[:, : W + 1, :].reshape(-1, D)
        for i in range(N_CORES)
    ]
    return np.concatenate(outs, axis=0)


# revision 10
# speedup vs baseline: 1.5449x; 1.2301x over previous
"""Trainium2 Bass kernel for BERT word-pooling (segment mean + CLS).

Computation (matches the jax reference):
  hidden = mean over 4 layers of hidden_layers[4, B, T, D]
  per example b: word_emb[j] = mean of hidden[b, t] over tokens with
  word_ids[b, t] == j (j < 100; 100 is the pad sentinel), empty words -> 0
  output rows per example: [cls = hidden[b, 0], word_emb[0..99]]
  -> [B*101, D]

Strategy: pure data parallel, 4 examples per core across 8 cores.
Per example the segment-sum is a one-hot matmul on the tensor engine:
  psum[j, d] = sum_{l,t} S[t, j] * h[l, t, d]      (layer sum folded in)
  counts[j]  = sum_t S[t, j] * 4.0
  out[j, d]  = psum[j, d] / max(counts[j], 4)      (= segment mean / 4 layers)
The one-hot columns are shifted by +1 (word j -> column j+1) and column 0
marks token 0, so the CLS row falls out of the same matmul + scale
pipeline (its count is 1 -> scale 1/4) and rows 0..100 of the result tile
are exactly one example's output block.

Memory plan (the kernel is HBM-bound: 32 MB of reads per core):
  * Token -> (partition, column) map is p = t // 4, c = t % 4, so every
    h load is 16 KB fully contiguous per partition (one big descriptor)
    and the word-id load is a tiny contiguous [128, 4] strided read.
  * All 16 layer-example h tiles live in SBUF at once (128 KB/partition
    in f16), so all 16 load DMAs are issued up front with no buffer-reuse
    dependencies: the SWDGE queue streams 32 MB at line rate start to
    finish while the PE/DVE chase layer arrivals.
  * f32 -> f16 happens inline in the SWDGE DMAs (only path that casts);
    all matmuls run f16 (4x fp32 PE rate), PSUM accumulates in f32.
  * Stores go on the scalar HWDGE queue (own queue -> a store waiting on
    compute can't head-of-line-block any load), 101 rows per example.
"""

import sys

for _p in ("/opt/trn_rl_repo", "/opt/trn_rl_repo/concourse"):
    if _p not in sys.path:
        sys.path.append(_p)

from contextlib import ExitStack

import numpy as np

import concourse.bacc as bacc
import concourse.bass as bass
import concourse.tile as tile
from concourse import mybir
from concourse.bass_utils import run_bass_kernel_spmd

B, T, D, W = 32, 512, 1024, 100
N_CORES = 8
BL = B // N_CORES          # examples per core
NT = T // 128              # token columns per partition (p = t // 4)
ND = D // 512              # 512-wide d chunks (one PSUM bank each)
OUT_PAD = 128              # padded per-example output rows: a 128-partition
                           # store spreads across all 16 SDMA engines, a
                           # 101-partition store lands on ONE engine (5 GB/s)
OUT_ROWS = BL * OUT_PAD    # output rows per core (kernel-side, padded)

_f32 = mybir.dt.float32
_f16 = mybir.dt.float16
_i32 = mybir.dt.int32


def _build_program() -> bass.Bass:
    # Bacc (not raw Bass): its compile() runs generate_event_semaphores,
    # which splits multi-wait DMAs (DMA instrs have a single HW wait slot).
    nc = bacc.Bacc(
        "TRN2", target_bir_lowering=False, debug=False, num_devices=N_CORES
    )
    hid = nc.declare_dram_parameter("hidden", [4, BL, T, D], _f32, isOutput=False)
    wid = nc.declare_dram_parameter("wid", [BL, T], _i32, isOutput=False)
    # f16 output (host upcasts): halves store traffic, and store-vs-load HBM
    # read/write mixing measurably slows the 32MB read stream
    out = nc.declare_dram_parameter("out", [OUT_ROWS, D], _f16, isOutput=True)

    with tile.TileContext(nc) as tc, ExitStack() as ctx:
        const = ctx.enter_context(tc.tile_pool(name="const", bufs=1))
        hres = ctx.enter_context(tc.tile_pool(name="hres", bufs=1))
        spool = ctx.enter_context(tc.tile_pool(name="spool", bufs=1))
        vpool = ctx.enter_context(tc.tile_pool(name="vpool", bufs=1))
        opool = ctx.enter_context(tc.tile_pool(name="opool", bufs=2))
        psum = ctx.enter_context(tc.tile_pool(name="psum", bufs=2, space="PSUM"))

        # column j holds value j-1 in every partition (f32: is_equal wants f32
        # operands). Word j then lands in one-hot column j+1, and column 0
        # (value -1, never a word id) is reserved for the CLS marker, so the
        # out_sb rows 0..100 are exactly one example's output block.
        iota_i = const.tile([128, 128], _i32)
        nc.gpsimd.iota(iota_i[:], [[1, 128]], base=-1, channel_multiplier=0)
        iota_t = const.tile([128, 128], _f32)
        nc.vector.tensor_copy(iota_t[:], iota_i[:])
        # counts rhs: 4.0 so counts come out as 4*count (the layer factor)
        ones4 = const.tile([128, 1], _f16)
        nc.vector.memset(ones4[:], 4.0)

        # word ids for all examples up front, one tiny contiguous read per
        # example: widt[p, 4b+c] = wid[b, 4p+c]. These ride the scalar
        # HWDGE queue, which is otherwise idle until the first store: on a
        # queue shared with the h stream their 16B packets would only get
        # one round-robin turn each and complete ~15 us late.
        widt = vpool.tile([128, BL * NT], _i32, tag="widt")
        for b in range(BL):
            nc.scalar.dma_start(
                widt[:, b * NT : (b + 1) * NT],
                wid[b].rearrange("(p c) -> p c", p=128),
            )
        widt_f = vpool.tile([128, BL * NT], _f32, tag="widt_f")
        nc.vector.tensor_copy(widt_f[:], widt[:])

        # ALL h loads issued before any compute: 16 x 2MB SWDGE DMAs with
        # inline f32->f16 cast into fully-resident tiles. Example-major so
        # example b's four layers land consecutively.
        h_tiles = [[None] * 4 for _ in range(BL)]
        for b in range(BL):
            for l in range(4):
                h_bl = hres.tile(
                    [128, NT, D], _f16, tag=f"h_{b}_{l}", name=f"h_{b}_{l}"
                )
                nc.gpsimd.dma_start(
                    h_bl[:], hid[l, b].rearrange("(p c) m -> p c m", p=128)
                )
                h_tiles[b][l] = h_bl

        # one-hot S tiles for all examples: S[p, j] = (wid[4p+c] == j-1)
        s_tiles = [[None] * NT for _ in range(BL)]
        for b in range(BL):
            for c in range(NT):
                s_c = spool.tile([128, 128], _f16, tag=f"s_{b}_{c}", name=f"s_{b}_{c}")
                nc.vector.tensor_scalar(
                    s_c[:], iota_t[:], widt_f[:, b * NT + c : b * NT + c + 1], None,
                    mybir.AluOpType.is_equal,
                )
                if c == 0:
                    # CLS marker: token 0 (p=0, c=0) also feeds output row 0
                    nc.vector.memset(s_c[0:1, 0:1], 1.0)
                s_tiles[b][c] = s_c

        # counts + reciprocal scales for all examples early (PE and DVE are
        # otherwise idle while the first h tiles stream in)
        recips = []
        for b in range(BL):
            counts_ps = psum.tile([128, 1], _f32, tag="counts", bufs=2)
            for c in range(NT):
                nc.tensor.matmul(
                    counts_ps[:], s_tiles[b][c][:], ones4[:],
                    start=(c == 0), stop=(c == NT - 1),
                )
            scale_t = vpool.tile([128, 1], _f32, tag=f"scale_{b}")
            recip_t = vpool.tile([128, 1], _f32, tag=f"recip_{b}")
            nc.vector.tensor_scalar_max(scale_t[:], counts_ps[:], 4.0)
            nc.vector.reciprocal(recip_t[:], scale_t[:])
            recips.append(recip_t)

        # per example: 32 accumulating f16 matmuls (4 layers x 4 columns x
        # 2 d-chunks), layer-major so compute chases the DMA arrival order.
        # Results stay resident in SBUF (f16) until the whole load stream is
        # done: overlapping stores with the read stream was measured to slow
        # the SDMA engines ~25% (HBM read/write mixing).
        out_sbs = []
        for b in range(BL):
            ps = [
                psum.tile([128, 512], _f32, tag=f"ps{d}", name=f"ps{d}", bufs=2)
                for d in range(ND)
            ]
            for l in range(4):
                for c in range(NT):
                    for d in range(ND):
                        nc.tensor.matmul(
                            ps[d][:],
                            s_tiles[b][c][:],
                            h_tiles[b][l][:, c, d * 512 : (d + 1) * 512],
                            start=(l == 0 and c == 0),
                            stop=(l == 3 and c == NT - 1),
                        )
            out_sb = opool.tile([128, D], _f16, tag=f"out_sb{b}", name=f"out_sb{b}")
            for d in range(ND):
                dsl = slice(d * 512, (d + 1) * 512)
                nc.vector.tensor_scalar(
                    out_sb[:, dsl], ps[d][:], recips[b][:, 0:1], None,
                    mybir.AluOpType.mult,
                )
            out_sbs.append(out_sb)

        # Stores go on the SAME gpsimd queue as the h loads, emitted after
        # them: queue FIFO order guarantees they drain only once the whole
        # 32MB read stream is done, so store writes never mix with (and slow
        # down) the HBM read stream. Full-128-partition stores (a
        # 101-partition store lands on a single SDMA engine at ~5 GB/s);
        # the host slices rows 0..100 of each example block.
        for b in range(BL):
            nc.gpsimd.dma_start(
                out[b * OUT_PAD : (b + 1) * OUT_PAD, :], out_sbs[b][:]
            )

    nc.compile()
    return nc


_PROGRAM = None
LAST_RESULTS = None   # BassKernelResults of the most recent run (for test.py)
TRACE = False         # set True from test.py to capture an NTFF profile


def _get_program() -> bass.Bass:
    global _PROGRAM
    if _PROGRAM is None:
        _PROGRAM = _build_program()
    return _PROGRAM


def kernel(hidden_layers, word_ids, num_words=W, **_ignored) -> np.ndarray:
    global LAST_RESULTS
    hidden_layers = np.asarray(hidden_layers, dtype=np.float32)
    word_ids = np.asarray(word_ids, dtype=np.int32)
    assert hidden_layers.shape == (4, B, T, D), hidden_layers.shape
    assert word_ids.shape == (B, T), word_ids.shape
    assert int(num_words) == W, num_words

    in_maps = []
    for i in range(N_CORES):
        sl = slice(i * BL, (i + 1) * BL)
        in_maps.append(
            {
                "hidden": np.ascontiguousarray(hidden_layers[:, sl]),
                "wid": np.ascontiguousarray(word_ids[sl]),
            }
        )

    res = run_bass_kernel_spmd(
        _get_program(), in_maps, core_ids=list(range(N_CORES)), trace=TRACE
    )
    LAST_RESULTS = res
    # kernel output is f16 padded to 128 rows per example; keep rows 0..100
    outs = [
        res.results[i]["out"].reshape(BL, OUT_PAD, D)[:, : W + 1, :].reshape(-1, D)
        for i in range(N_CORES)
    ]
    return np.concatenate(outs, axis=0).astype(np.float32)


# revision 12
# speedup vs baseline: 1.5857x; 1.0264x over previous
"""Trainium2 Bass kernel for BERT word-pooling (segment mean + CLS).

Computation (matches the jax reference):
  hidden = mean over 4 layers of hidden_layers[4, B, T, D]
  per example b: word_emb[j] = mean of hidden[b, t] over tokens with
  word_ids[b, t] == j (j < 100; 100 is the pad sentinel), empty words -> 0
  output rows per example: [cls = hidden[b, 0], word_emb[0..99]]
  -> [B*101, D]

Strategy: pure data parallel, 4 examples per core across 8 cores.
Per example the segment-sum is a one-hot matmul on the tensor engine:
  psum[j, d] = sum_{l,t} S[t, j] * h[l, t, d]      (layer sum folded in)
  counts[j]  = sum_t S[t, j] * 4.0
  out[j, d]  = psum[j, d] / max(counts[j], 4)      (= segment mean / 4 layers)
The one-hot columns are shifted by +1 (word j -> column j+1) and column 0
marks token 0, so the CLS row falls out of the same matmul + scale
pipeline (its count is 1 -> scale 1/4) and rows 0..100 of the result tile
are exactly one example's output block.

Memory plan (the kernel is HBM-bound: 32 MB of reads per core):
  * Token -> (partition, column) map is p = t // 4, c = t % 4, so every
    h load is 16 KB fully contiguous per partition (one big descriptor)
    and the word-id load is a tiny contiguous [128, 4] strided read.
  * All 16 layer-example h tiles live in SBUF at once (128 KB/partition
    in f16), so all 16 load DMAs are issued up front with no buffer-reuse
    dependencies: the SWDGE queue streams 32 MB at line rate start to
    finish while the PE/DVE chase layer arrivals.
  * f32 -> f16 happens inline in the SWDGE DMAs (only path that casts);
    all matmuls run f16 (4x fp32 PE rate), PSUM accumulates in f32.
  * Stores go on the scalar HWDGE queue (own queue -> a store waiting on
    compute can't head-of-line-block any load), 101 rows per example.
"""

import sys

for _p in ("/opt/trn_rl_repo", "/opt/trn_rl_repo/concourse"):
    if _p not in sys.path:
        sys.path.append(_p)

from contextlib import ExitStack

import numpy as np

import concourse.bacc as bacc
import concourse.bass as bass
import concourse.tile as tile
from concourse import mybir
from concourse.bass_utils import run_bass_kernel_spmd

B, T, D, W = 32, 512, 1024, 100
N_CORES = 8
BL = B // N_CORES          # examples per core
NT = T // 128              # token columns per partition (p = t // 4)
ND = D // 512              # 512-wide d chunks (one PSUM bank each)
OUT_PAD = 128              # padded per-example output rows: a 128-partition
                           # store spreads across all 16 SDMA engines, a
                           # 101-partition store lands on ONE engine (5 GB/s)
OUT_ROWS = BL * OUT_PAD    # output rows per core (kernel-side, padded)

_f32 = mybir.dt.float32
_f16 = mybir.dt.float16
_i32 = mybir.dt.int32


def _build_program() -> bass.Bass:
    # Bacc (not raw Bass): its compile() runs generate_event_semaphores,
    # which splits multi-wait DMAs (DMA instrs have a single HW wait slot).
    nc = bacc.Bacc(
        "TRN2", target_bir_lowering=False, debug=False, num_devices=N_CORES
    )
    hid = nc.declare_dram_parameter("hidden", [4, BL, T, D], _f32, isOutput=False)
    wid = nc.declare_dram_parameter("wid", [BL, T], _i32, isOutput=False)
    # f16 output (host upcasts): halves store traffic, and store-vs-load HBM
    # read/write mixing measurably slows the 32MB read stream
    out = nc.declare_dram_parameter("out", [OUT_ROWS, D], _f16, isOutput=True)

    with tile.TileContext(nc) as tc, ExitStack() as ctx:
        const = ctx.enter_context(tc.tile_pool(name="const", bufs=1))
        hres = ctx.enter_context(tc.tile_pool(name="hres", bufs=1))
        spool = ctx.enter_context(tc.tile_pool(name="spool", bufs=1))
        vpool = ctx.enter_context(tc.tile_pool(name="vpool", bufs=1))
        opool = ctx.enter_context(tc.tile_pool(name="opool", bufs=2))
        psum = ctx.enter_context(tc.tile_pool(name="psum", bufs=2, space="PSUM"))

        # word ids for all examples up front, one tiny contiguous read per
        # example: widt[p, 4b+c] = wid[b, 4p+c]. These ride the scalar
        # HWDGE queue, which is otherwise idle: on a queue shared with the
        # h stream their 16B packets would only get one round-robin turn
        # each and complete ~15 us late.
        widt = vpool.tile([128, BL * NT], _i32, tag="widt")
        for b in range(BL):
            nc.scalar.dma_start(
                widt[:, b * NT : (b + 1) * NT],
                wid[b].rearrange("(p c) -> p c", p=128),
            )
        widt_f = vpool.tile([128, BL * NT], _f32, tag="widt_f")
        nc.vector.tensor_copy(widt_f[:], widt[:])

        # ALL h loads issued before any compute: 2MB SWDGE DMAs with inline
        # f32->f16 cast into fully-resident tiles. Example-major so example
        # b's four layers land consecutively. The very last layer-example is
        # split into 4 chunk DMAs so its matmuls chase the chunks and only
        # one (c, d) pair of matmuls remains after the final bytes land.
        h_tiles = [[None] * 4 for _ in range(BL)]
        iota_i = const.tile([128, 128], _i32)
        for b in range(BL):
            for l in range(4):
                h_bl = hres.tile(
                    [128, NT, D], _f16, tag=f"h_{b}_{l}", name=f"h_{b}_{l}"
                )
                src = hid[l, b].rearrange("(p c) m -> p c m", p=128)
                if b == BL - 1 and l == 3:
                    for c in range(NT):
                        nc.gpsimd.dma_start(h_bl[:, c, :], src[:, c, :])
                else:
                    nc.gpsimd.dma_start(h_bl[:], src)
                h_tiles[b][l] = h_bl
                if b == 0 and l == 0:
                    # iota for the one-hot columns: column j holds value j-1
                    # in every partition (f32: is_equal wants f32 operands).
                    # Word j lands in one-hot column j+1; column 0 (value -1,
                    # never a word id) is reserved for the CLS marker. Issued
                    # after the first h DMA so the stream starts sooner.
                    nc.gpsimd.iota(
                        iota_i[:], [[1, 128]], base=-1, channel_multiplier=0
                    )
        iota_t = const.tile([128, 128], _f32)
        nc.vector.tensor_copy(iota_t[:], iota_i[:])
        # counts rhs: 4.0 so counts come out as 4*count (the layer factor)
        ones4 = const.tile([128, 1], _f16)
        nc.vector.memset(ones4[:], 4.0)

        # one-hot S tiles for all examples: S[p, j] = (wid[4p+c] == j-1)
        s_tiles = [[None] * NT for _ in range(BL)]
        for b in range(BL):
            for c in range(NT):
                s_c = spool.tile([128, 128], _f16, tag=f"s_{b}_{c}", name=f"s_{b}_{c}")
                nc.vector.tensor_scalar(
                    s_c[:], iota_t[:], widt_f[:, b * NT + c : b * NT + c + 1], None,
                    mybir.AluOpType.is_equal,
                )
                if c == 0:
                    # CLS marker: token 0 (p=0, c=0) also feeds output row 0
                    nc.vector.memset(s_c[0:1, 0:1], 1.0)
                s_tiles[b][c] = s_c

        # counts + reciprocal scales for all examples early (PE and DVE are
        # otherwise idle while the first h tiles stream in)
        recips = []
        for b in range(BL):
            counts_ps = psum.tile([128, 1], _f32, tag="counts", bufs=2)
            for c in range(NT):
                nc.tensor.matmul(
                    counts_ps[:], s_tiles[b][c][:], ones4[:],
                    start=(c == 0), stop=(c == NT - 1),
                )
            scale_t = vpool.tile([128, 1], _f32, tag=f"scale_{b}")
            recip_t = vpool.tile([128, 1], _f32, tag=f"recip_{b}")
            nc.vector.tensor_scalar_max(scale_t[:], counts_ps[:], 4.0)
            nc.vector.reciprocal(recip_t[:], scale_t[:])
            recips.append(recip_t)

        # per example: 32 accumulating f16 matmuls (4 layers x 4 columns x
        # 2 d-chunks), layer-major so compute chases the DMA arrival order.
        # Results stay resident in SBUF (f16) until the whole load stream is
        # done: overlapping stores with the read stream was measured to slow
        # the SDMA engines ~25% (HBM read/write mixing).
        out_sbs = []
        for b in range(BL):
            ps = [
                psum.tile([128, 512], _f32, tag=f"ps{d}", name=f"ps{d}", bufs=2)
                for d in range(ND)
            ]
            for l in range(4):
                for c in range(NT):
                    for d in range(ND):
                        nc.tensor.matmul(
                            ps[d][:],
                            s_tiles[b][c][:],
                            h_tiles[b][l][:, c, d * 512 : (d + 1) * 512],
                            start=(l == 0 and c == 0),
                            stop=(l == 3 and c == NT - 1),
                        )
            out_sb = opool.tile([128, D], _f16, tag=f"out_sb{b}", name=f"out_sb{b}")
            for d in range(ND):
                dsl = slice(d * 512, (d + 1) * 512)
                nc.vector.tensor_scalar(
                    out_sb[:, dsl], ps[d][:], recips[b][:, 0:1], None,
                    mybir.AluOpType.mult,
                )
            out_sbs.append(out_sb)

        # Stores go on the SAME gpsimd queue as the h loads, emitted after
        # them: queue FIFO order guarantees they drain only once the whole
        # 32MB read stream is done, so store writes never mix with (and slow
        # down) the HBM read stream. Full-128-partition stores (a
        # 101-partition store lands on a single SDMA engine at ~5 GB/s);
        # the host slices rows 0..100 of each example block. Split per
        # d-chunk so each half fires as soon as its scale is done.
        for b in range(BL):
            for d in range(ND):
                dsl = slice(d * 512, (d + 1) * 512)
                nc.gpsimd.dma_start(
                    out[b * OUT_PAD : (b + 1) * OUT_PAD, dsl], out_sbs[b][:, dsl]
                )

    nc.compile()
    return nc


_PROGRAM = None
LAST_RESULTS = None   # BassKernelResults of the most recent run (for test.py)
TRACE = False         # set True from test.py to capture an NTFF profile


def _get_program() -> bass.Bass:
    global _PROGRAM
    if _PROGRAM is None:
        _PROGRAM = _build_program()
    return _PROGRAM


def kernel(hidden_layers, word_ids, num_words=W, **_ignored) -> np.ndarray:
    global LAST_RESULTS
    hidden_layers = np.asarray(hidden_layers, dtype=np.float32)
    word_ids = np.asarray(word_ids, dtype=np.int32)
    assert hidden_layers.shape == (4, B, T, D), hidden_layers.shape
    assert word_ids.shape == (B, T), word_ids.shape
    assert int(num_words) == W, num_words

    in_maps = []
    for i in range(N_CORES):
        sl = slice(i * BL, (i + 1) * BL)
        in_maps.append(
            {
                "hidden": np.ascontiguousarray(hidden_layers[:, sl]),
                "wid": np.ascontiguousarray(word_ids[sl]),
            }
        )

    res = run_bass_kernel_spmd(
        _get_program(), in_maps, core_ids=list(range(N_CORES)), trace=TRACE
    )
    LAST_RESULTS = res
    # kernel output is f16 padded to 128 rows per example; keep rows 0..100
    outs = [
        res.results[i]["out"].reshape(BL, OUT_PAD, D)[:, : W + 1, :].reshape(-1, D)
        for i in range(N_CORES)
    ]
    return np.concatenate(outs, axis=0).astype(np.float32)
